# revision 25
# baseline (speedup 1.0000x reference)
"""Trainium2 Bass kernel for nn_AdultConnectome (gnn_message_passing).

Computes y = A^L @ x for a COO sparse adjacency A (100000 nodes, 3.2M edges),
x [100000, 512] fp32, L = layer_number hops.

Distribution: 8 NeuronCores; core c owns the column-node block
[12544*c, 12544*(c+1)) and ALL 512 features (bf16). Edges are partitioned by
their source (col) node block, so every per-edge gather index is block-local
(< 12544, fits the gather DMA's int16 index limit). Each hop:

  1. dma_gather: per edge e, fetch h[col_local[e], :] (512 bf16 = 1KB rows)
     from the core-local table in HBM. Edges are pre-sorted by destination
     row; one gather per 128-row destination block (nchunk_pb*128 idx slots,
     real edges first, -1 padding at the tail). A per-core count tensor is
     loaded into a Pool register per gather (num_idxs_reg) so padded slots
     generate no descriptors and move no bytes. Gathers round-robin across
     4 SWDGE queues so descriptor generation pipelines.
  2. For each 128-edge chunk, load the host-precomputed scatter matrix
     P[e, r] = w[e] * (row_local[e] == r) (bf16, streamed from HBM) and
     accumulate PSUM[r, f] += P^T @ G on TensorE. This is the segment-sum.
     (Padded slots keep stale gather-buffer bf16 data; their P columns are
     zero, so they contribute nothing. Buffers are memset once at start.)
  3. Evict each 128-row block to a [100352, 512] bf16 partial in HBM.
  4. ReduceScatter(add) over all 8 cores sums the partials and hands core c
     its own 12544-row block for the next hop's gather table.

All structure (chunk counts, padding) is computed host-side from the actual
edge data and baked into the compiled graph; it is identical on all 8 cores
(SPMD), with per-core differences only in input tensors (idx, counts, P).
"""

import numpy as np
import ml_dtypes

import concourse.bass as bass
import concourse.bacc as bacc
import concourse.tile as tile
import concourse.mybir as mybir
from concourse.bass_utils import run_bass_kernel_spmd

BF16 = ml_dtypes.bfloat16

N_CORES = 8
P = 128
N_NODES = 100000
N_FEAT = 512
NB = 12544                 # nodes per core block (100352 = 8 * 12544)
NPAD = NB * N_CORES        # 100352
NRB = NPAD // P            # 784 row blocks
NSEG = 7                   # ReduceScatter slabs per hop (98 = 7*14)
NQ = 4                     # SWDGE queues; gathers round-robin across them


def _prep_core(rows, cols, ws, core):
    """Per-core edge preprocessing."""
    lo, hi = NB * core, NB * (core + 1)
    m = (cols >= lo) & (cols < hi)
    r = rows[m]
    c = (cols[m] - lo).astype(np.int64)
    w = ws[m]
    order = np.argsort(r, kind="stable")
    r, c, w = r[order], c[order], w[order]
    rb = r >> 7
    rl = (r & 127).astype(np.int64)
    cnt = np.bincount(rb, minlength=NRB)
    return r, c, w, rb, rl, cnt


def _block_seq():
    """Row-block processing order: segment-major (q, core, i) so that each
    of the NSEG ReduceScatter slabs covers a contiguous run of processed
    blocks and can be issued while later segments still compute."""
    bpc = NRB // N_CORES              # 98 blocks per core block
    bps = bpc // NSEG                 # 14 blocks per (segment, core)
    seq = []
    for q in range(NSEG):
        for cc in range(N_CORES):
            for i in range(bps):
                seq.append(cc * bpc + q * bps + i)
    return np.array(seq, dtype=np.int64)


def _pack_core(r, c, w, rb, rl, cnt, nchunk_pb):
    """Pack one core's edges into padded device arrays (idx + P tiles +
    per-gather valid counts). One gather per row-block position: gpg =
    nchunk_pb*128 idx slots, real edges first, -1 padding at the tail."""
    epb = nchunk_pb * P               # padded edges per row block = gpg
    tot = NRB * epb
    ncht = NRB * nchunk_pb
    bs = _block_seq()
    posof = np.empty(NRB, dtype=np.int64)
    posof[bs] = np.arange(NRB)
    # within each block position, order edges by source col so gather
    # descriptors walk ascending HBM addresses (better bank spread); the
    # P tile encodes each slot's destination row, so any order is valid
    order2 = np.lexsort((c, posof[rb]))
    r, c, w, rb, rl = r[order2], c[order2], w[order2], rb[order2], rl[order2]
    pos = posof[rb]
    cnt_seq = cnt[bs]
    col_pad = np.full(tot, -1, dtype=np.int16)
    starts = np.zeros(NRB, dtype=np.int64)
    starts[1:] = np.cumsum(cnt_seq)[:-1]
    j_within = np.arange(len(r)) - starts[pos]
    slot = pos * epb + j_within
    col_pad[slot] = c.astype(np.int16)

    # P tiles: logically [ncht, 128, 128] bf16; P[k, p, rl] = w for edge
    # (k*128+p). Device layout groups the nchunk_pb chunks of one position
    # with partition-major rows so one plain 2D DMA per gather lands them in
    # SBUF: [NRB*128, nchunk_pb*128].
    p_tiles = np.zeros(ncht * P * P, dtype=BF16)
    chunk = slot // P
    part = slot % P
    p_tiles[chunk * (P * P) + part * P + rl] = w.astype(BF16)
    p_tiles = (p_tiles.reshape(NRB, nchunk_pb, P, P)
               .transpose(0, 2, 1, 3).reshape(NRB * P, nchunk_pb * P))

    # gather idx layout: per gather (= position) of gpg idx, wrapped
    # [16, gpg/16], tiled to 128 partitions; gathers concat along free dim
    gpg = epb
    idx_grp = col_pad.reshape(NRB, gpg // 16, 16)
    idx_wrapped = idx_grp.transpose(0, 2, 1)
    idx_dev = np.tile(idx_wrapped, (1, 8, 1))
    idx_dev = np.concatenate(idx_dev, axis=1)
    return {
        "gidx": np.ascontiguousarray(idx_dev),
        "ptiles": np.ascontiguousarray(p_tiles),
        "gcnt": np.ascontiguousarray(
            cnt_seq.astype(np.int32).reshape(1, NRB)),
    }


def _build_graph(n_hops, nchunk_pb):
    """Build the SPMD Bass graph (identical for all cores)."""
    gpg = nchunk_pb * P               # idx slots per gather (one row block)
    idx_cols = NRB * (gpg // 16)
    gcols = gpg // 16

    nc = bacc.Bacc("TRN2", target_bir_lowering=False, debug=False,
                   num_devices=N_CORES, num_swdge_queues=NQ)

    h0_in = nc.dram_tensor("h0", [NB, N_FEAT], mybir.dt.bfloat16,
                           kind="ExternalInput")
    gidx_in = nc.dram_tensor("gidx", [P, idx_cols], mybir.dt.int16,
                             kind="ExternalInput")
    pt_in = nc.dram_tensor("ptiles", [NRB * P, gpg], mybir.dt.bfloat16,
                           kind="ExternalInput")
    cnt_in = nc.dram_tensor("gcnt", [1, NRB], mybir.dt.int32,
                            kind="ExternalInput")
    y_out = nc.dram_tensor("y", [NB, N_FEAT], mybir.dt.bfloat16,
                           kind="ExternalOutput")

    with tile.TileContext(nc) as tc:
        with tc.tile_pool(name="sbuf", bufs=8) as sbuf, \
             tc.tile_pool(name="sbuf_idx", bufs=1) as sbuf_idx, \
             tc.tile_pool(name="psum", bufs=8, space="PSUM") as psum, \
             tc.tile_pool(name="dram", bufs=2, space="DRAM") as dram:

            # hop-invariant inputs, loaded once
            idx_t = sbuf_idx.tile([P, idx_cols], mybir.dt.int16, tag="idx")
            nc.sync.dma_start(idx_t[:], gidx_in.ap()[:, :])
            cnt_t = sbuf_idx.tile([1, NRB], mybir.dt.int32, tag="cnt")
            nc.sync.dma_start(cnt_t[:], cnt_in.ap()[:, :])

            # gather output buffers hold stale data in skipped (padded)
            # slots; zero them once so the first rotations are finite.
            for zi in range(8):
                zt = sbuf.tile([P, nchunk_pb, N_FEAT], mybir.dt.bfloat16,
                               tag="gath", name=f"zg{zi}")
                nc.vector.memset(zt[:], 0.0)

            cnt_reg = nc.alloc_register(mybir.EngineType.Pool)

            h_tabs = [h0_in.ap()[:, :]]
            for hop in range(n_hops):
                partial = dram.tile([NPAD, N_FEAT], mybir.dt.bfloat16,
                                    tag="partial")
                h_tab = h_tabs[hop]
                g_list = [None] * NRB
                p_list = [None] * NRB

                def issue_gather(pos, h_tab=h_tab, g_list=g_list,
                                 p_list=p_list):
                    g_t = sbuf.tile([P, nchunk_pb, N_FEAT],
                                    mybir.dt.bfloat16, tag="gath")
                    nc.gpsimd.reg_load(cnt_reg, cnt_t[0:1, pos:pos + 1])
                    nc.gpsimd.dma_gather(
                        out_ap=g_t[:],
                        in_ap=h_tab,
                        idxs_ap=idx_t[:, pos * gcols:(pos + 1) * gcols],
                        num_idxs=gpg,
                        num_idxs_reg=cnt_reg,
                        elem_size=N_FEAT,
                        queue_num=pos % NQ,
                    )
                    g_list[pos] = g_t
                    # P tiles for this position's chunks, loaded via the
                    # ScalarE HWDGE ring so they don't queue behind the
                    # Sync-ring evict/idx DMAs
                    p_t = sbuf.tile([P, nchunk_pb, P], mybir.dt.bfloat16,
                                    tag="ptile")
                    nc.scalar.dma_start(
                        p_t[:], pt_in.ap()[pos * P:(pos + 1) * P, :])
                    p_list[pos] = p_t

                h_next = dram.tile([NB, N_FEAT], mybir.dt.bfloat16,
                                   tag="hnext")
                bpseg = NRB // NSEG        # 112 processed blocks per slab
                rseg = NB // NSEG          # 1792 h_next rows per slab
                rs_next = 0

                def maybe_issue_rs(done_blocks, force=False,
                                   partial=partial, h_next=h_next):
                    nonlocal rs_next
                    while rs_next < NSEG:
                        need = (rs_next + 1) * bpseg + 128
                        if not force and done_blocks < min(need, NRB):
                            break
                        if not force and rs_next == NSEG - 1:
                            break
                        j = rs_next
                        nc.gpsimd.collective_compute(
                            "ReduceScatter",
                            mybir.AluOpType.add,
                            replica_groups=[list(range(N_CORES))],
                            ins=[partial[j * bpseg * P:(j + 1) * bpseg * P,
                                         :].opt()],
                            outs=[h_next[j * rseg:(j + 1) * rseg, :].opt()],
                        )
                        rs_next += 1

                for pos in range(NRB):
                    issue_gather(pos)
                    ps = psum.tile([P, N_FEAT], mybir.dt.float32,
                                   space="PSUM", tag="ps")
                    for cch in range(nchunk_pb):
                        nc.tensor.matmul(
                            out=ps[:],
                            lhsT=p_list[pos][:, cch, :],
                            rhs=g_list[pos][:, cch, :],
                            start=(cch == 0),
                            stop=(cch == nchunk_pb - 1),
                        )
                    ev = sbuf.tile([P, N_FEAT], mybir.dt.bfloat16,
                                   tag="evict")
                    nc.vector.tensor_copy(ev[:], ps[:])
                    nc.sync.dma_start(
                        partial[pos * P:(pos + 1) * P, :], ev[:])
                    maybe_issue_rs(pos + 1)
                maybe_issue_rs(NRB, force=True)
                h_tabs.append(h_next[:])

            nc.sync.dma_start(y_out.ap()[:, :], h_tabs[n_hops])

    nc.compile()
    return nc


_GRAPH_CACHE = {}


def kernel(x, weights, row, col, layer_number):
    x = np.asarray(x)
    weights = np.asarray(weights)
    rows = np.asarray(row).astype(np.int64)
    cols = np.asarray(col).astype(np.int64)
    n_hops = int(layer_number)
    if n_hops == 0:
        return x.astype(np.float32)

    preps = [_prep_core(rows, cols, weights, c) for c in range(N_CORES)]
    nchunk_pb = max(int(np.ceil(p[5].max() / P)) for p in preps)
    nchunk_pb = max(nchunk_pb, 1)

    key = (n_hops, nchunk_pb)
    if key not in _GRAPH_CACHE:
        _GRAPH_CACHE[key] = _build_graph(n_hops, nchunk_pb)
    nc = _GRAPH_CACHE[key]

    x_pad = np.zeros((NPAD, N_FEAT), dtype=np.float32)
    x_pad[:N_NODES] = x
    x_bf = x_pad.astype(BF16)

    in_maps = []
    for c in range(N_CORES):
        dev = _pack_core(*preps[c], nchunk_pb)
        in_maps.append({
            "h0": np.ascontiguousarray(x_bf[NB * c:NB * (c + 1)]),
            "gidx": dev["gidx"],
            "ptiles": dev["ptiles"],
            "gcnt": dev["gcnt"],
        })

    res = run_bass_kernel_spmd(nc, in_maps, core_ids=list(range(N_CORES)))
    y = np.concatenate([res.results[c]["y"].astype(np.float32)
                        for c in range(N_CORES)], axis=0)
    return y[:N_NODES]
